# revision 11
# baseline (speedup 1.0000x reference)
"""LIF spike (leaky integrate-and-fire with hard reset) Trainium2 kernel.

x: [B=32, T=16, C=128, H=32, W=32] f32  ->  spikes, same shape.
Per element (b,c,h,w), sequential over t:
    v = mem*TAU + x_t ; s = (v >= TH) ; mem = v * (v < TH)

Sharding: batch dim B=32 split across 8 NeuronCores (4 per core), pure
data-parallel SPMD (no collectives).

Per-core pipeline (v2): the 4 local b's are processed as 2 groups of 2
([C=128 partitions, 2*H*W=2048 free] tiles) so the two groups pipeline
against each other through PSUM double-buffering.

Per (t, group) step:
    PE : v_psum = tauI.T @ mem  (start)  +  I.T @ x  (accumulate)
         -- diagonal matmuls turn the AXPY into TensorEngine work
    ACT: v_sb = copy(v_psum)             (ScalarE, frees PSUM + gives DVE
                                          an SBUF source for 2x perf mode)
    DVE: s_u8 = (v_sb >= TH)             (tensor_scalar, uint8 spike out)
    DVE: mem  = (v_sb < TH) * v_sb       (fused hard reset)
    DMA: store s_u8 (uint8 -> 1/4 the write traffic; host casts to f32)
"""

import sys

import numpy as np

for _p in ("/opt/trn_rl_repo",):
    if _p not in sys.path:
        sys.path.insert(0, _p)

import concourse.bacc as bacc
import concourse.bass as bass
import concourse.mybir as mybir
from concourse.bass_utils import run_bass_kernel_spmd
from concourse.masks import make_identity
from concourse.tile import TileContext

B, T, C, H, W = 32, 16, 128, 32, 32
HW = H * W
N_CORES = 8
BL = B // N_CORES  # 4 batches per core
GB = 2  # batches per group
NG = BL // GB  # 2 groups
GF = GB * HW  # 2048 free elems per group tile
TAU = 0.25
TH = 0.5

OUT_DT = mybir.dt.float32
OUT_NP = np.float32

_nc_cache = None


def _build_nc():
    nc = bacc.Bacc(
        "TRN2", target_bir_lowering=False, debug=False, num_devices=N_CORES
    )
    x = nc.dram_tensor("x", [BL, T, C, HW], mybir.dt.float32, kind="ExternalInput")
    s = nc.dram_tensor("s", [BL, T, C, HW], OUT_DT, kind="ExternalOutput")

    with TileContext(nc) as tc:
        with (
            tc.tile_pool(name="const", bufs=1) as cp,
            tc.tile_pool(name="mem", bufs=1) as mp,
            tc.tile_pool(name="xin", bufs=4) as xp,
            tc.tile_pool(name="vsb", bufs=3) as vp,
            tc.tile_pool(name="sout", bufs=4) as sp,
            tc.tile_pool(name="psum", bufs=2, space="PSUM") as pp,
        ):
            # Diagonal weight matrices: tauI (decay) and I (accumulate x).
            tau_eye = cp.tile([C, C], mybir.dt.float32, tag="tau_eye")
            nc.gpsimd.memset(tau_eye[:], 0.0)
            nc.gpsimd.affine_select(
                out=tau_eye[:],
                in_=tau_eye[:],
                compare_op=mybir.AluOpType.not_equal,
                fill=TAU,
                base=0,
                pattern=[[-1, C]],
                channel_multiplier=1,
            )
            eye = cp.tile([C, C], mybir.dt.float32, tag="eye")
            make_identity(nc, eye[:])

            mems = []
            for g in range(NG):
                m = mp.tile([C, GF], mybir.dt.float32, tag=f"mem{g}")
                nc.vector.memset(m[:], 0.0)
                mems.append(m)

            for t in range(T):
                for g in range(NG):
                    b0 = g * GB
                    xt = xp.tile([C, GF], mybir.dt.float32, tag="x")
                    for bb in range(GB):
                        nc.sync.dma_start(
                            out=xt[:, bb * HW : (bb + 1) * HW],
                            in_=x[b0 + bb, t],
                        )
                    m = mems[g]
                    v_ps = pp.tile([C, GF], mybir.dt.float32, tag="v")
                    # v = tau*mem  (diagonal matmul, per 512-wide PSUM bank)
                    for j in range(0, GF, 512):
                        nc.tensor.matmul(
                            v_ps[:, j : j + 512],
                            tau_eye[:],
                            m[:, j : j + 512],
                            start=True,
                            stop=False,
                        )
                    # v += x
                    for j in range(0, GF, 512):
                        nc.tensor.matmul(
                            v_ps[:, j : j + 512],
                            eye[:],
                            xt[:, j : j + 512],
                            start=False,
                            stop=True,
                        )
                    v_sb = vp.tile([C, GF], mybir.dt.float32, tag="v_sb")
                    nc.scalar.copy(v_sb[:], v_ps[:])
                    st = sp.tile([C, GF], OUT_DT, tag="s")
                    # s = (v >= TH)
                    nc.vector.tensor_scalar(
                        out=st[:],
                        in0=v_sb[:],
                        scalar1=TH,
                        scalar2=None,
                        op0=mybir.AluOpType.is_ge,
                    )
                    # mem = (v < TH) * v    (hard reset)
                    nc.vector.scalar_tensor_tensor(
                        out=m[:],
                        in0=v_sb[:],
                        scalar=TH,
                        in1=v_sb[:],
                        op0=mybir.AluOpType.is_lt,
                        op1=mybir.AluOpType.mult,
                    )
                    for bb in range(GB):
                        nc.sync.dma_start(
                            out=s[b0 + bb, t],
                            in_=st[:, bb * HW : (bb + 1) * HW],
                        )
    nc.compile()
    return nc


def _get_nc():
    global _nc_cache
    if _nc_cache is None:
        _nc_cache = _build_nc()
    return _nc_cache


def _ensure_ntff_hook():
    """Install the antenv.axon_hooks shim so trace=True works under axon.

    The agent image's antenv package lacks axon_hooks; build the same
    ctypes-based hook trn_agent_boot would have registered.
    """
    import types

    try:
        from antenv import axon_hooks  # noqa: F401

        return
    except ImportError:
        pass
    import antenv
    from trn_agent_boot.trn_boot import _ntff_profile_via_ctypes

    hook = _ntff_profile_via_ctypes("/opt/axon/libaxon_pjrt.so")
    mod = types.ModuleType("antenv.axon_hooks")
    holder = {"hook": hook}
    mod.set_axon_ntff_profile_hook = lambda h: holder.__setitem__("hook", h)
    mod.get_axon_ntff_profile_hook = lambda: holder["hook"]
    sys.modules["antenv.axon_hooks"] = mod
    antenv.axon_hooks = mod


def kernel(x: np.ndarray, _trace: bool = False, **_unused):
    assert x.shape == (B, T, C, H, W), x.shape
    if _trace:
        _ensure_ntff_hook()
    xr = np.ascontiguousarray(x, dtype=np.float32).reshape(B, T, C, HW)
    nc = _get_nc()
    in_maps = [{"x": xr[i * BL : (i + 1) * BL]} for i in range(N_CORES)]
    res = run_bass_kernel_spmd(
        nc, in_maps, core_ids=list(range(N_CORES)), trace=_trace
    )
    out = np.concatenate([r["s"] for r in res.results], axis=0)
    out = out.reshape(B, T, C, H, W).astype(np.float32)
    if _trace:
        kernel.last_results = res
    return out


# revision 14
# speedup vs baseline: 1.3825x; 1.3825x over previous
"""LIF spike (leaky integrate-and-fire with hard reset) Trainium2 kernel.

x: [B=32, T=16, C=128, H=32, W=32] f32  ->  spikes, same shape.
Per element (b,c,h,w), sequential over t:
    v = mem*TAU + x_t ; s = (v >= TH) ; mem = v * (v < TH)

Sharding: batch dim B=32 split across 8 NeuronCores (4 per core), pure
data-parallel SPMD (no collectives).

Per-core pipeline (v3): all 4 local b's form one [C=128, 4*H*W=4096] tile.
Per timestep:
    DVE: v   = (mem * TAU) + x      (fused scalar_tensor_tensor, in-place)
    ACT: sig = Sign(v - TH)         (ScalarE LUT; exact; fp8 output)
    DVE: mem = (v < TH) * v         (fused hard reset)
The spike is stored as sign(v-TH) in fp8e4 (1 byte, values -1/0/+1): the
write traffic drops 4x vs f32 and the host decodes spike = (sign bit
clear).  Sign outputs accumulate 8 timesteps per batch in SBUF so the
store DMAs are 1 MB each, into a [BL, C, T*HW] DRAM layout that keeps
per-partition bytes contiguous; the host transposes back to [B,T,C,H,W].
"""

import sys

import numpy as np

for _p in ("/opt/trn_rl_repo",):
    if _p not in sys.path:
        sys.path.insert(0, _p)

import concourse.bacc as bacc
import concourse.bass as bass
import concourse.mybir as mybir
from concourse.bass_utils import run_bass_kernel_spmd
from concourse.tile import TileContext

B, T, C, H, W = 32, 16, 128, 32, 32
HW = H * W
N_CORES = 8
BL = B // N_CORES  # 4 batches per core
GF = BL * HW  # 4096: all local batches in one tile's free dim
TH_HALF = T // 2  # timesteps per store chunk
TAU = 0.25
TH = 0.5

_nc_cache = None


def _build_nc():
    nc = bacc.Bacc(
        "TRN2", target_bir_lowering=False, debug=False, num_devices=N_CORES
    )
    x = nc.dram_tensor("x", [BL, T, C, HW], mybir.dt.float32, kind="ExternalInput")
    s = nc.dram_tensor("s", [BL, C, T * HW], mybir.dt.float8e4, kind="ExternalOutput")

    with TileContext(nc) as tc:
        with (
            tc.tile_pool(name="const", bufs=1) as cp,
            tc.tile_pool(name="mem", bufs=1) as mp,
            tc.tile_pool(name="xin", bufs=4) as xp,
            tc.tile_pool(name="sacc", bufs=2) as sp,
        ):
            neg_th = cp.tile([C, 1], mybir.dt.float32, tag="neg_th")
            nc.vector.memset(neg_th[:], -TH)
            m = mp.tile([C, GF], mybir.dt.float32, tag="mem")
            nc.vector.memset(m[:], 0.0)

            sacc = None
            for t in range(T):
                th = t % TH_HALF
                if th == 0:
                    # [c, b, t_half, hw] fp8 accumulator for one 8-step chunk
                    sacc = sp.tile(
                        [C, BL, TH_HALF, HW], mybir.dt.float8e4, tag="sacc"
                    )
                xt = xp.tile([C, GF], mybir.dt.float32, tag="x")
                for b in range(BL):
                    nc.sync.dma_start(
                        out=xt[:, b * HW : (b + 1) * HW], in_=x[b, t]
                    )
                v = xt[:]
                # v = mem*TAU + x_t   (in place over the x tile)
                nc.vector.scalar_tensor_tensor(
                    out=v,
                    in0=m[:],
                    scalar=TAU,
                    in1=v,
                    op0=mybir.AluOpType.mult,
                    op1=mybir.AluOpType.add,
                )
                # sig = Sign(v - TH): -1 below threshold, 0/+1 at/above
                nc.scalar.sign(
                    out=sacc[:, :, th, :],
                    in_=v.rearrange("c (b f) -> c b f", b=BL),
                    bias=neg_th[:],
                )
                # mem = (v < TH) * v    (hard reset)
                nc.vector.scalar_tensor_tensor(
                    out=m[:],
                    in0=v,
                    scalar=TH,
                    in1=v,
                    op0=mybir.AluOpType.is_lt,
                    op1=mybir.AluOpType.mult,
                )
                if th == TH_HALF - 1:
                    half = t // TH_HALF
                    for b in range(BL):
                        nc.sync.dma_start(
                            out=s[
                                b,
                                :,
                                half * TH_HALF * HW : (half + 1) * TH_HALF * HW,
                            ],
                            in_=sacc[:, b],
                        )
    nc.compile()
    return nc


def _get_nc():
    global _nc_cache
    if _nc_cache is None:
        _nc_cache = _build_nc()
    return _nc_cache


def _ensure_ntff_hook():
    """Install the antenv.axon_hooks shim so trace=True works under axon.

    The agent image's antenv package lacks axon_hooks; build the same
    ctypes-based hook trn_agent_boot would have registered.
    """
    import types

    try:
        from antenv import axon_hooks  # noqa: F401

        return
    except ImportError:
        pass
    import antenv
    from trn_agent_boot.trn_boot import _ntff_profile_via_ctypes

    hook = _ntff_profile_via_ctypes("/opt/axon/libaxon_pjrt.so")
    mod = types.ModuleType("antenv.axon_hooks")
    holder = {"hook": hook}
    mod.set_axon_ntff_profile_hook = lambda h: holder.__setitem__("hook", h)
    mod.get_axon_ntff_profile_hook = lambda: holder["hook"]
    sys.modules["antenv.axon_hooks"] = mod
    antenv.axon_hooks = mod


def kernel(x: np.ndarray, _trace: bool = False, **_unused):
    assert x.shape == (B, T, C, H, W), x.shape
    if _trace:
        _ensure_ntff_hook()
    xr = np.ascontiguousarray(x, dtype=np.float32).reshape(B, T, C, HW)
    nc = _get_nc()
    in_maps = [{"x": xr[i * BL : (i + 1) * BL]} for i in range(N_CORES)]
    res = run_bass_kernel_spmd(
        nc, in_maps, core_ids=list(range(N_CORES)), trace=_trace
    )
    # decode: fp8 sign values -> spike = 1 where sign bit clear (v >= TH)
    outs = []
    for r in res.results:
        raw = np.asarray(r["s"]).view(np.uint8).reshape(BL, C, T, HW)
        outs.append(raw < 0x80)
    out = np.concatenate(outs, axis=0)  # [B, C, T, HW] bool
    out = out.transpose(0, 2, 1, 3).astype(np.float32).reshape(B, T, C, H, W)
    if _trace:
        kernel.last_results = res
    return out


# revision 17
# speedup vs baseline: 1.4624x; 1.0578x over previous
"""LIF spike (leaky integrate-and-fire with hard reset) Trainium2 kernel.

x: [B=32, T=16, C=128, H=32, W=32] f32  ->  spikes, same shape.
Per element (b,c,h,w), sequential over t:
    v = mem*TAU + x_t ; s = (v >= TH) ; mem = v * (v < TH)

Sharding: batch dim B=32 split across 8 NeuronCores (4 per core), pure
data-parallel SPMD (no collectives).

Per-core pipeline (v3): all 4 local b's form one [C=128, 4*H*W=4096] tile.
Per timestep:
    DVE: v   = (mem * TAU) + x      (fused scalar_tensor_tensor, in-place)
    ACT: sig = Sign(v - TH)         (ScalarE LUT; exact; fp8 output)
    DVE: mem = (v < TH) * v         (fused hard reset)
The spike is stored as sign(v-TH) in fp8e4 (1 byte, values -1/0/+1): the
write traffic drops 4x vs f32 and the host decodes spike = (sign bit
clear).  Sign outputs accumulate 8 timesteps per batch in SBUF so the
store DMAs are 1 MB each, into a [BL, C, T*HW] DRAM layout that keeps
per-partition bytes contiguous; the host transposes back to [B,T,C,H,W].
"""

import sys

import numpy as np

for _p in ("/opt/trn_rl_repo",):
    if _p not in sys.path:
        sys.path.insert(0, _p)

import concourse.bacc as bacc
import concourse.bass as bass
import concourse.mybir as mybir
from concourse.bass_utils import run_bass_kernel_spmd
from concourse.tile import TileContext

B, T, C, H, W = 32, 16, 128, 32, 32
HW = H * W
N_CORES = 8
BL = B // N_CORES  # 4 batches per core
GF = BL * HW  # 4096: all local batches in one tile's free dim
TCH = 4  # timesteps per store chunk
TAU = 0.25
TH = 0.5

_nc_cache = None


def _build_nc():
    nc = bacc.Bacc(
        "TRN2", target_bir_lowering=False, debug=False, num_devices=N_CORES
    )
    x = nc.dram_tensor("x", [BL, T, C, HW], mybir.dt.float32, kind="ExternalInput")
    s = nc.dram_tensor("s", [BL, C, T * HW], mybir.dt.float8e4, kind="ExternalOutput")

    with TileContext(nc) as tc:
        with (
            tc.tile_pool(name="const", bufs=1) as cp,
            tc.tile_pool(name="mem", bufs=1) as mp,
            tc.tile_pool(name="xin", bufs=6) as xp,
            tc.tile_pool(name="sacc", bufs=3) as sp,
        ):
            neg_th = cp.tile([C, 1], mybir.dt.float32, tag="neg_th")
            nc.vector.memset(neg_th[:], -TH)
            m = mp.tile([C, GF], mybir.dt.float32, tag="mem")

            sacc = None
            for t in range(T):
                th = t % TCH
                if th == 0:
                    # [c, b, t_chunk, hw] fp8 accumulator for one chunk
                    sacc = sp.tile(
                        [C, BL, TCH, HW], mybir.dt.float8e4, tag="sacc"
                    )
                xt = xp.tile([C, GF], mybir.dt.float32, tag="x")
                for b in range(BL):
                    nc.sync.dma_start(
                        out=xt[:, b * HW : (b + 1) * HW], in_=x[b, t]
                    )
                v = xt[:]
                if t > 0:
                    # v = mem*TAU + x_t   (in place over the x tile)
                    nc.vector.scalar_tensor_tensor(
                        out=v,
                        in0=m[:],
                        scalar=TAU,
                        in1=v,
                        op0=mybir.AluOpType.mult,
                        op1=mybir.AluOpType.add,
                    )
                # else: mem is 0, so v = x_t as loaded.
                # sig = Sign(v - TH): -1 below threshold, 0/+1 at/above
                nc.scalar.sign(
                    out=sacc[:, :, th, :],
                    in_=v.rearrange("c (b f) -> c b f", b=BL),
                    bias=neg_th[:],
                )
                if t < T - 1:
                    # mem = (v < TH) * v    (hard reset; last step's mem unused)
                    nc.vector.scalar_tensor_tensor(
                        out=m[:],
                        in0=v,
                        scalar=TH,
                        in1=v,
                        op0=mybir.AluOpType.is_lt,
                        op1=mybir.AluOpType.mult,
                    )
                if th == TCH - 1:
                    chunk = t // TCH
                    for b in range(BL):
                        nc.sync.dma_start(
                            out=s[
                                b,
                                :,
                                chunk * TCH * HW : (chunk + 1) * TCH * HW,
                            ],
                            in_=sacc[:, b],
                        )
    nc.compile()
    return nc


def _get_nc():
    global _nc_cache
    if _nc_cache is None:
        _nc_cache = _build_nc()
    return _nc_cache


def _ensure_ntff_hook():
    """Install the antenv.axon_hooks shim so trace=True works under axon.

    The agent image's antenv package lacks axon_hooks; build the same
    ctypes-based hook trn_agent_boot would have registered.
    """
    import types

    try:
        from antenv import axon_hooks  # noqa: F401

        return
    except ImportError:
        pass
    import antenv
    from trn_agent_boot.trn_boot import _ntff_profile_via_ctypes

    hook = _ntff_profile_via_ctypes("/opt/axon/libaxon_pjrt.so")
    mod = types.ModuleType("antenv.axon_hooks")
    holder = {"hook": hook}
    mod.set_axon_ntff_profile_hook = lambda h: holder.__setitem__("hook", h)
    mod.get_axon_ntff_profile_hook = lambda: holder["hook"]
    sys.modules["antenv.axon_hooks"] = mod
    antenv.axon_hooks = mod


def kernel(x: np.ndarray, _trace: bool = False, **_unused):
    assert x.shape == (B, T, C, H, W), x.shape
    if _trace:
        _ensure_ntff_hook()
    xr = np.ascontiguousarray(x, dtype=np.float32).reshape(B, T, C, HW)
    nc = _get_nc()
    in_maps = [{"x": xr[i * BL : (i + 1) * BL]} for i in range(N_CORES)]
    res = run_bass_kernel_spmd(
        nc, in_maps, core_ids=list(range(N_CORES)), trace=_trace
    )
    # decode: fp8 sign values -> spike = 1 where sign bit clear (v >= TH)
    outs = []
    for r in res.results:
        raw = np.asarray(r["s"]).view(np.uint8).reshape(BL, C, T, HW)
        outs.append(raw < 0x80)
    out = np.concatenate(outs, axis=0)  # [B, C, T, HW] bool
    out = out.transpose(0, 2, 1, 3).astype(np.float32).reshape(B, T, C, H, W)
    if _trace:
        kernel.last_results = res
    return out


# revision 19
# speedup vs baseline: 1.4688x; 1.0044x over previous
"""LIF spike (leaky integrate-and-fire with hard reset) Trainium2 kernel.

x: [B=32, T=16, C=128, H=32, W=32] f32  ->  spikes, same shape.
Per element (b,c,h,w), sequential over t:
    v = mem*TAU + x_t ; s = (v >= TH) ; mem = v * (v < TH)

Sharding: batch dim B=32 split across 8 NeuronCores (4 per core), pure
data-parallel SPMD (no collectives).

Per-core pipeline (v3): all 4 local b's form one [C=128, 4*H*W=4096] tile.
Per timestep:
    DVE: v   = (mem * TAU) + x      (fused scalar_tensor_tensor, in-place)
    ACT: sig = Sign(v - TH)         (ScalarE LUT; exact; fp8 output)
    DVE: mem = (v < TH) * v         (fused hard reset)
The spike is stored as sign(v-TH) in fp8e4 (1 byte, values -1/0/+1): the
write traffic drops 4x vs f32 and the host decodes spike = (sign bit
clear).  Sign outputs accumulate 8 timesteps per batch in SBUF so the
store DMAs are 1 MB each, into a [BL, C, T*HW] DRAM layout that keeps
per-partition bytes contiguous; the host transposes back to [B,T,C,H,W].
"""

import sys

import numpy as np

for _p in ("/opt/trn_rl_repo",):
    if _p not in sys.path:
        sys.path.insert(0, _p)

import concourse.bacc as bacc
import concourse.bass as bass
import concourse.mybir as mybir
from concourse.bass_utils import run_bass_kernel_spmd
from concourse.tile import TileContext

B, T, C, H, W = 32, 16, 128, 32, 32
HW = H * W
N_CORES = 8
BL = B // N_CORES  # 4 batches per core
GF = BL * HW  # 4096: all local batches in one tile's free dim
TCH = 4  # timesteps per store chunk
TAU = 0.25
TH = 0.5

_nc_cache = None


def _build_nc():
    nc = bacc.Bacc(
        "TRN2", target_bir_lowering=False, debug=False, num_devices=N_CORES
    )
    x = nc.dram_tensor("x", [BL, T, C, HW], mybir.dt.float32, kind="ExternalInput")
    s = nc.dram_tensor("s", [BL, C, T * HW], mybir.dt.float8e4, kind="ExternalOutput")

    with TileContext(nc) as tc:
        with (
            tc.tile_pool(name="const", bufs=1) as cp,
            tc.tile_pool(name="mem", bufs=1) as mp,
            tc.tile_pool(name="xin", bufs=6) as xp,
            tc.tile_pool(name="sacc", bufs=3) as sp,
        ):
            neg_th = cp.tile([C, 1], mybir.dt.float32, tag="neg_th")
            nc.vector.memset(neg_th[:], -TH)
            m = mp.tile([C, GF], mybir.dt.float32, tag="mem")

            sacc = None
            for t in range(T):
                th = t % TCH
                if th == 0:
                    # [c, b, t_chunk, hw] fp8 accumulator for one chunk
                    sacc = sp.tile(
                        [C, BL, TCH, HW], mybir.dt.float8e4, tag="sacc"
                    )
                xt = xp.tile([C, GF], mybir.dt.float32, tag="x")
                for b in range(BL):
                    dma_eng = nc.sync if b % 2 == 0 else nc.scalar
                    dma_eng.dma_start(
                        out=xt[:, b * HW : (b + 1) * HW], in_=x[b, t]
                    )
                v = xt[:]
                if t > 0:
                    # v = mem*TAU + x_t   (in place over the x tile)
                    nc.vector.scalar_tensor_tensor(
                        out=v,
                        in0=m[:],
                        scalar=TAU,
                        in1=v,
                        op0=mybir.AluOpType.mult,
                        op1=mybir.AluOpType.add,
                    )
                # else: mem is 0, so v = x_t as loaded.
                # sig = Sign(v - TH): -1 below threshold, 0/+1 at/above
                chunk = t // TCH
                if t == T - 1:
                    # last step: per-b sign so each store launches immediately
                    for b in range(BL):
                        nc.scalar.sign(
                            out=sacc[:, b, th, :],
                            in_=v[:, b * HW : (b + 1) * HW],
                            bias=neg_th[:],
                        )
                        nc.sync.dma_start(
                            out=s[
                                b,
                                :,
                                chunk * TCH * HW : (chunk + 1) * TCH * HW,
                            ],
                            in_=sacc[:, b],
                        )
                else:
                    nc.scalar.sign(
                        out=sacc[:, :, th, :],
                        in_=v.rearrange("c (b f) -> c b f", b=BL),
                        bias=neg_th[:],
                    )
                if t < T - 1:
                    # mem = (v < TH) * v    (hard reset; last step's mem unused)
                    nc.vector.scalar_tensor_tensor(
                        out=m[:],
                        in0=v,
                        scalar=TH,
                        in1=v,
                        op0=mybir.AluOpType.is_lt,
                        op1=mybir.AluOpType.mult,
                    )
                    if th == TCH - 1:
                        for b in range(BL):
                            nc.sync.dma_start(
                                out=s[
                                    b,
                                    :,
                                    chunk * TCH * HW : (chunk + 1) * TCH * HW,
                                ],
                                in_=sacc[:, b],
                            )
    nc.compile()
    return nc


def _get_nc():
    global _nc_cache
    if _nc_cache is None:
        _nc_cache = _build_nc()
    return _nc_cache


def _ensure_ntff_hook():
    """Install the antenv.axon_hooks shim so trace=True works under axon.

    The agent image's antenv package lacks axon_hooks; build the same
    ctypes-based hook trn_agent_boot would have registered.
    """
    import types

    try:
        from antenv import axon_hooks  # noqa: F401

        return
    except ImportError:
        pass
    import antenv
    from trn_agent_boot.trn_boot import _ntff_profile_via_ctypes

    hook = _ntff_profile_via_ctypes("/opt/axon/libaxon_pjrt.so")
    mod = types.ModuleType("antenv.axon_hooks")
    holder = {"hook": hook}
    mod.set_axon_ntff_profile_hook = lambda h: holder.__setitem__("hook", h)
    mod.get_axon_ntff_profile_hook = lambda: holder["hook"]
    sys.modules["antenv.axon_hooks"] = mod
    antenv.axon_hooks = mod


def kernel(x: np.ndarray, _trace: bool = False, **_unused):
    assert x.shape == (B, T, C, H, W), x.shape
    if _trace:
        _ensure_ntff_hook()
    xr = np.ascontiguousarray(x, dtype=np.float32).reshape(B, T, C, HW)
    nc = _get_nc()
    in_maps = [{"x": xr[i * BL : (i + 1) * BL]} for i in range(N_CORES)]
    res = run_bass_kernel_spmd(
        nc, in_maps, core_ids=list(range(N_CORES)), trace=_trace
    )
    # decode: fp8 sign values -> spike = 1 where sign bit clear (v >= TH)
    outs = []
    for r in res.results:
        raw = np.asarray(r["s"]).view(np.uint8).reshape(BL, C, T, HW)
        outs.append(raw < 0x80)
    out = np.concatenate(outs, axis=0)  # [B, C, T, HW] bool
    out = out.transpose(0, 2, 1, 3).astype(np.float32).reshape(B, T, C, H, W)
    if _trace:
        kernel.last_results = res
    return out
